# revision 16
# baseline (speedup 1.0000x reference)
"""Trainium2 Bass kernel for nn_CudaFastWeightSumTwoLinearTransformerLayer.

Restructured fast-weight (linear attention) transformer layer:
  out_t = tril(Q K^T) V with denom_t = cumsum(K)_t . q_t, where
  q pre-scaling folds SCALE / (d0 + EPS*qs) into Q before the A-matmul so
  the attention output needs no post-normalization.

Sharding: data-parallel over batch. B=64 -> 8 cores x 8 batch elements.
Each core runs the full layer on its batch shard; no collectives.

Self-contained: hardcodes all shapes; imports only the runtime environment
(/opt/trn_rl_repo) and numpy.
"""

import sys
import numpy as np

for _p in ("/opt/trn_rl_repo",):
    if _p not in sys.path:
        sys.path.insert(0, _p)

import concourse.bass as bass  # noqa: E402
import concourse.bacc as bacc  # noqa: E402
import concourse.mybir as mybir  # noqa: E402
from concourse import tile  # noqa: E402
from concourse.bass_utils import run_bass_kernel_spmd  # noqa: E402

F32 = mybir.dt.float32
F32R = mybir.dt.float32r
AF = mybir.ActivationFunctionType
OP = mybir.AluOpType

L, B, NH, DH, DM = 256, 64, 8, 64, 512
NC = 8            # cores
BC = B // NC      # batch per core = 8
SCALE = 1.0 / DH ** 0.5
EPS = 1e-5
P = 128


def _consts():
    tri = (np.arange(P)[:, None] <= np.arange(P)[None, :]).astype(np.float32)
    ones = np.ones((P, P), np.float32)
    ident = np.eye(P, dtype=np.float32)
    return tri, ones, ident


def build_program(apply_affine: bool):
    """Build the per-core Bass program (SPMD, identical on all cores)."""
    nc = bacc.Bacc("TRN2", target_bir_lowering=False, debug=False)

    hT = nc.dram_tensor("hT", [BC, DM, L], F32R, kind="ExternalInput").ap()
    hres = nc.dram_tensor("hres", [BC, L, DM], F32, kind="ExternalInput").ap()
    wT = nc.dram_tensor("wT", [DM, 4 * NH * DH], F32R, kind="ExternalInput").ap()
    owT = nc.dram_tensor("owT", [NH * DH, DM], F32R, kind="ExternalInput").ap()
    gates = nc.dram_tensor("gates", [P, 32], F32, kind="ExternalInput").ap()
    if apply_affine:
        lng = nc.dram_tensor("lng", [P, DM], F32, kind="ExternalInput").ap()
        lnb = nc.dram_tensor("lnb", [P, DM], F32, kind="ExternalInput").ap()
    out = nc.dram_tensor("out", [BC, L, DM], F32, kind="ExternalOutput").ap()

    tri_d = nc.dram_tensor("tri_c", [P, P], F32R, kind="ExternalInput").ap()
    ones_d = nc.dram_tensor("ones_c", [P, P], F32R, kind="ExternalInput").ap()
    ident_d = nc.dram_tensor("ident_c", [P, P], F32R, kind="ExternalInput").ap()

    with tile.TileContext(nc) as tc:
        _emit(tc, nc, hT, hres, wT, owT, gates, out,
              tri_d, ones_d, ident_d,
              lng if apply_affine else None,
              lnb if apply_affine else None)
    nc.compile()
    return nc


def _emit(tc, nc, hT, hres, wT, owT, gates, out, tri_d, ones_d, ident_d,
          lng, lnb):
    import contextlib
    ctx = contextlib.ExitStack()
    with ctx:
        const = ctx.enter_context(tc.tile_pool(name="const", bufs=1))
        # --- persistent constants ---
        wT_sb = []
        for i in range(4):
            t = const.tile([P, 4 * NH * DH], F32R, name=f"wTs{i}")
            nc.sync.dma_start(t[:], wT[i * P:(i + 1) * P, :])
            wT_sb.append(t)
        owT_sb = []
        for i in range(4):
            t = const.tile([P, DM], F32R, name=f"owTs{i}")
            nc.sync.dma_start(t[:], owT[i * P:(i + 1) * P, :])
            owT_sb.append(t)
        tri_sb = const.tile([P, P], F32R, name="tri_sb")
        nc.sync.dma_start(tri_sb[:], tri_d[:])
        ones_sb = const.tile([P, P], F32R, name="ones_sb")
        nc.sync.dma_start(ones_sb[:], ones_d[:])
        ident_sb = const.tile([P, P], F32R, name="ident_sb")
        nc.sync.dma_start(ident_sb[:], ident_d[:])
        gates_sb = const.tile([P, 32], F32, name="gates_sb")
        nc.sync.dma_start(gates_sb[:], gates[:])
        if lng is not None:
            lng_sb = const.tile([P, DM], F32, name="lng_sb")
            nc.sync.dma_start(lng_sb[:], lng[:])
            lnb_sb = const.tile([P, DM], F32, name="lnb_sb")
            nc.sync.dma_start(lnb_sb[:], lnb[:])

        # --- pools ---
        pq = ctx.enter_context(tc.tile_pool(name="pq", bufs=3, space="PSUM"))
        ptr = ctx.enter_context(tc.tile_pool(name="ptr", bufs=2, space="PSUM"))
        pat = ctx.enter_context(tc.tile_pool(name="pat", bufs=2, space="PSUM"))
        po2 = ctx.enter_context(tc.tile_pool(name="po2", bufs=1, space="PSUM"))

        sp_h = ctx.enter_context(tc.tile_pool(name="sp_h", bufs=8))
        sp_hres = ctx.enter_context(tc.tile_pool(name="sp_hres", bufs=4))
        sp_phi = ctx.enter_context(tc.tile_pool(name="sp_phi", bufs=4))
        sp_v = ctx.enter_context(tc.tile_pool(name="sp_v", bufs=4))
        sp_ka = ctx.enter_context(tc.tile_pool(name="sp_ka", bufs=4))
        sp_qt = ctx.enter_context(tc.tile_pool(name="sp_qt", bufs=4))
        sp_scr = ctx.enter_context(tc.tile_pool(name="sp_scr", bufs=4))
        sp_ktq = ctx.enter_context(tc.tile_pool(name="sp_ktq", bufs=6))
        sp_atm = ctx.enter_context(tc.tile_pool(name="sp_atm", bufs=6))
        sp_ot = ctx.enter_context(tc.tile_pool(name="sp_ot", bufs=4))
        sp_x = ctx.enter_context(tc.tile_pool(name="sp_x", bufs=3))
        sp_xsq = ctx.enter_context(tc.tile_pool(name="sp_xsq", bufs=2))
        sp_o = ctx.enter_context(tc.tile_pool(name="sp_o", bufs=3))
        sp_s = ctx.enter_context(tc.tile_pool(name="sp_s", bufs=10))

        # per-b state passed from qkv stage to attn stage
        state = {}

        def emit_qkv(b):
            h_tiles = []
            for i in range(4):
                t = sp_h.tile([P, L], F32R, name="h_t", tag="h_t")
                nc.sync.dma_start(t[:], hT[b, i * P:(i + 1) * P, :])
                h_tiles.append(t)
            hr = []
            for tcn in range(2):
                t = sp_hres.tile([P, DM], F32, name="hres_t", tag="hres_t")
                nc.sync.dma_start(t[:], hres[b, tcn * P:(tcn + 1) * P, :])
                hr.append(t)

            phi = []   # [128, 1536] per tc: phi(q|k1|k2) head-major triples
            vall = []  # [128, 512] per tc: raw v, head-major
            for tcn in range(2):
                ph = sp_phi.tile([P, 1536], F32, name="phi_t", tag="phi_t")
                va = sp_v.tile([P, 1024], F32R, name="v_t", tag="v_t")
                for ec in range(4):   # psum bank = 2 heads x (q k1 k2 v)
                    pqt = pq.tile([P, DM], F32, name="pq_t", tag="pq_t")
                    for dmc in range(4):
                        nc.tensor.matmul(
                            pqt[:],
                            h_tiles[dmc][:, tcn * P:(tcn + 1) * P],
                            wT_sb[dmc][:, ec * DM:(ec + 1) * DM],
                            start=(dmc == 0), stop=(dmc == 3))
                    bank3 = pqt[:].rearrange("p (h c) -> p h c", h=2)
                    stripes = bank3[:, :, 0:192]
                    e_sl = ph[:, ec * 384:(ec + 1) * 384].rearrange(
                        "p (h c) -> p h c", h=2)
                    # e = exp(x) (ACT, reads psum)
                    nc.scalar.activation(e_sl, stripes, AF.Exp)
                    # e = min(e, 1)  (DVE, sbuf 1-input)
                    nc.vector.tensor_scalar_min(e_sl, e_sl, 1.0)
                    # phi = max(x,0) + e  (DVE, psum + sbuf)
                    nc.vector.scalar_tensor_tensor(
                        e_sl, stripes, 0.0, e_sl, OP.max, OP.add)
                    # evac raw v (ACT) into padded layout [Ve |0| Vo]
                    v_sl = va[:, ec * 256:(ec + 1) * 256].rearrange(
                        "p (g d) -> p g d", g=4)[:, 0:4:3, :]
                    nc.scalar.activation(v_sl, bank3[:, :, 192:256], AF.Copy)
                    nc.gpsimd.memset(
                        va[:, ec * 256 + 64:ec * 256 + 192].bitcast(F32), 0.0)
                phi.append(ph)
                vall.append(va)
            state[b] = (phi, vall, hr)

        import os as _os
        _stage = int(_os.environ.get("KERNEL_STAGE", "7"))

        def _dump(b, ap):
            nc.sync.dma_start(out[b, 0:P, :], ap)

        def emit_attn(b):
            phi, vall, hr = state.pop(b)
            if _stage == 1:
                _dump(b, phi[0][:, 0:DM])
                return
            # --- sums over d per (h, comp): 2-hop reduction ---
            sums = []   # [128, 24] per tc ; cols h-major (q,k1,k2)
            for tcn in range(2):
                ss = sp_scr.tile([P, 768], F32, name="ssum_t", tag="ssum_t", bufs=2)
                ph4 = phi[tcn][:].rearrange("p (g t w) -> p g t w", g=24, t=2)
                nc.vector.tensor_add(
                    ss[:].rearrange("p (g w) -> p g w", g=24),
                    ph4[:, :, 0, :], ph4[:, :, 1, :])
                s24 = sp_s.tile([P, 24], F32, name="s24_t", tag="s24_t")
                nc.vector.tensor_reduce(
                    s24[:], ss[:].rearrange("p (g w) -> p g w", g=24),
                    mybir.AxisListType.X, OP.add)
                sums.append(s24)

            # --- gated k combination ---
            ka = []
            for tcn in range(2):
                s3 = sums[tcn][:].rearrange("p (h c) -> p h c", h=NH)
                rk = sp_s.tile([P, 16], F32, name="rk_t", tag="rk_t")
                rk2 = rk[:].rearrange("p (h c) -> p h c", h=NH)
                nc.vector.reciprocal(rk2, s3[:, :, 1:3])
                ck = sp_s.tile([P, 16], F32, name="ck_t", tag="ck_t")
                ck2 = ck[:].rearrange("p (h c) -> p h c", h=NH)
                g4 = gates_sb[:].rearrange("p (t h c) -> p t h c", t=2, h=NH)
                nc.vector.tensor_mul(ck2, rk2, g4[:, tcn])
                ph4 = phi[tcn][:].rearrange("p (h c d) -> p h c d", h=NH, c=3)
                kt = sp_ka.tile([P, DM], F32R, name="ka_t", tag="ka_t")
                kt3 = kt[:].rearrange("p (h d) -> p h d", h=NH)
                scr = sp_scr.tile([P, DM], F32, name="kscr_t", tag="kscr_t")
                scr3 = scr[:].rearrange("p (h d) -> p h d", h=NH)
                # scr = phi_k2 * c2   (GPSIMD)
                nc.gpsimd.tensor_tensor(
                    scr3, ph4[:, :, 2, :],
                    ck2[:, :, 1:2].broadcast_to([P, NH, DH]), OP.mult)
                # kt = phi_k1 * c1    (DVE)
                nc.vector.tensor_tensor(
                    kt3, ph4[:, :, 1, :],
                    ck2[:, :, 0:1].broadcast_to([P, NH, DH]), OP.mult)
                nc.vector.tensor_add(kt3, kt3, scr3)
                ka.append(kt)
            if _stage == 2:
                _dump(b, ka[0][:].bitcast(F32))
                return

            # --- D = cumsum(k) via triangular matmuls ---
            dps = []
            dp0 = pq.tile([P, DM], F32, name="dp_t", tag="pq_t")
            nc.tensor.matmul(dp0[:], tri_sb[:],
                             ka[0][:], start=True, stop=True)
            dps.append(dp0)
            dp1 = pq.tile([P, DM], F32, name="dp_t", tag="pq_t")
            nc.tensor.matmul(dp1[:], ones_sb[:],
                             ka[0][:], start=True, stop=False)
            nc.tensor.matmul(dp1[:], tri_sb[:],
                             ka[1][:], start=False, stop=True)
            dps.append(dp1)

            # --- alpha & q-tilde ---
            qts = []
            for tcn in range(2):
                ph4 = phi[tcn][:].rearrange("p (h c d) -> p h c d", h=NH, c=3)
                phq = ph4[:, :, 0, :]
                prod = sp_scr.tile([P, DM], F32, name="prod_t", tag="kscr_t")
                nc.vector.tensor_tensor(
                    prod[:].rearrange("p (h d) -> p h d", h=NH),
                    dps[tcn][:].rearrange("p (h d) -> p h d", h=NH),
                    phq, OP.mult)
                p4 = prod[:].rearrange("p (h t w) -> p h t w", h=NH, t=2)
                ph32 = sp_scr.tile([P, 256], F32, name="ph32_t", tag="ph32_t")
                nc.vector.tensor_add(
                    ph32[:].rearrange("p (h w) -> p h w", h=NH),
                    p4[:, :, 0, :], p4[:, :, 1, :])
                d08 = sp_s.tile([P, NH], F32, name="d08_t", tag="d08_t")
                nc.vector.tensor_reduce(
                    d08[:], ph32[:].rearrange("p (h w) -> p h w", h=NH),
                    mybir.AxisListType.X, OP.add)
                # w8 = qs*EPS + d0 ; r8 = 1/w8 ; r8s = r8*SCALE
                s3 = sums[tcn][:].rearrange("p (h c) -> p h c", h=NH)
                w8 = sp_s.tile([P, NH], F32, name="w8_t", tag="w8_t")
                w82 = w8[:].rearrange("p (h c) -> p h c", h=NH)
                nc.vector.scalar_tensor_tensor(
                    w82, s3[:, :, 0:1], EPS,
                    d08[:].rearrange("p (h c) -> p h c", h=NH),
                    OP.mult, OP.add)
                r8 = sp_s.tile([P, NH], F32, name="r8_t", tag="r8_t")
                nc.vector.reciprocal(r8[:], w8[:])
                r8s = sp_s.tile([P, NH], F32, name="r8s_t", tag="r8s_t")
                nc.vector.tensor_scalar(r8s[:], r8[:], SCALE, None, OP.mult)
                qt = sp_qt.tile([P, DM], F32R, name="qt_t", tag="qt_t")
                nc.vector.tensor_tensor(
                    qt[:].rearrange("p (h d) -> p h d", h=NH), phq,
                    r8s[:].rearrange("p (h c) -> p h c", h=NH).broadcast_to(
                        [P, NH, DH]),
                    OP.mult)
                qts.append(qt)
            if _stage == 3:
                _dump(b, qts[0][:].bitcast(F32))
                return

            # --- transposes: per head [128,64]->[64,128]; pack cols so all
            # matmul operands stay at partition offset 0 ---
            ktq = []   # [64, 1024] per pair: head-even [qT|kT], head-odd [qT|kT]
            for p_ in range(4):
                khp = sp_ktq.tile([64, 1024], F32R, name="ktq_t", tag="ktq_t")
                for hh in range(2):
                    h = 2 * p_ + hh
                    pt = ptr.tile([64, DM], F32R, name="ptr_t", tag="ptr_t")
                    for tcn in range(2):
                        nc.tensor.transpose(
                            pt[:, tcn * P:(tcn + 1) * P],
                            qts[tcn][:, h * DH:(h + 1) * DH], ident_sb[:])
                        nc.tensor.transpose(
                            pt[:, 256 + tcn * P:256 + (tcn + 1) * P],
                            ka[tcn][:, h * DH:(h + 1) * DH], ident_sb[:])
                    nc.scalar.activation(
                        khp[:, hh * DM:(hh + 1) * DM], pt[:], AF.Copy)
                ktq.append(khp)
            if _stage == 4:
                _dump(b, ktq[0][:].bitcast(F32))
                return

            # --- mm1: AT = K qt^T per (s-chunk, head pair) + mask ---
            atm = {}   # (sc, pair) -> sbuf [128, 512] (2 heads x [t0|t1])
            for p_ in range(4):
                for sc in range(2):
                    pa = pat.tile([P, DM], F32, name="pat_t", tag="pat_t")
                    for hh in range(2):
                        base = hh * DM
                        nc.tensor.matmul(
                            pa[:, hh * 256:(hh + 1) * 256],
                            ktq[p_][0:DH,
                                    base + 256 + sc * P:
                                    base + 256 + (sc + 1) * P],
                            ktq[p_][0:DH, base:base + 256],
                            start=True, stop=True)
                    am = sp_atm.tile([P, DM], F32R, name="atm_t", tag="atm_t")
                    am3 = am[:].rearrange("p (h t) -> p h t", h=2)
                    pa3 = pa[:].rearrange("p (h t) -> p h t", h=2)
                    trib = tri_sb[:].bitcast(F32).rearrange(
                        "p (o t) -> p o t", o=1).broadcast_to([P, 2, P])
                    if sc == 0:
                        # diag tri-mask on t0 halves (DVE), plain copy t1 (ACT)
                        nc.vector.tensor_tensor(
                            am3[:, :, 0:P], pa3[:, :, 0:P], trib, OP.mult)
                        nc.scalar.activation(
                            am3[:, :, P:256], pa3[:, :, P:256], AF.Copy)
                    else:
                        # t0 halves are anti-causal -> zero; tri-mask t1
                        nc.gpsimd.memset(
                            am[:].bitcast(F32).rearrange(
                                "p (h t) -> p h t", h=2)[:, :, 0:P], 0.0)
                        nc.vector.tensor_tensor(
                            am3[:, :, P:256], pa3[:, :, P:256], trib, OP.mult)
                    atm[(sc, p_)] = am
            if _stage == 5:
                _dump(b, atm[(0, 0)][:].bitcast(F32))
                return

            # --- mm2: OT = V^T ATm; padded V blocks keep dst partitions 0 ---
            ots = []
            for pp in range(2):
                po = po2.tile([P, DM], F32, name="po2_t", tag="po2_t")
                for pi in range(2):
                    p_ = 2 * pp + pi
                    col = pi * 256
                    first = True
                    for hh in range(2):
                        for sc in range(2):
                            nc.tensor.matmul(
                                po[:, col:col + 256],
                                vall[sc][:, p_ * 256 + hh * P:
                                         p_ * 256 + (hh + 1) * P],
                                atm[(sc, p_)][:, hh * 256:(hh + 1) * 256],
                                start=first, stop=(hh == 1 and sc == 1))
                            first = False
                for pi in range(2):
                    ot = sp_ot.tile([P, 256], F32R, name="ot_t", tag="ot_t")
                    nc.scalar.activation(ot[:], po[:, pi * 256:(pi + 1) * 256],
                                         AF.Copy)
                    ots.append(ot)
            if _stage == 6:
                _dump(b, ots[0][:].bitcast(F32))
                _dump2 = sp_o.tile([P, 256], F32, name="d2_t", tag="xo_t")
                return

            # --- o_proj + residual + layernorm ---
            for tcn in range(2):
                pat_o = pq.tile([P, DM], F32, name="po_t", tag="pq_t")
                for p_ in range(4):
                    nc.tensor.matmul(
                        pat_o[:],
                        ots[p_][:, tcn * P:(tcn + 1) * P],
                        owT_sb[p_][:],
                        start=(p_ == 0), stop=(p_ == 3))
                x = sp_x.tile([P, DM], F32, name="x_t", tag="x_t")
                s1 = sp_s.tile([P, 1], F32, name="s1_t", tag="s1_t")
                nc.vector.scalar_tensor_tensor(
                    x[:], hr[tcn][:], 1.0, pat_o[:], OP.mult, OP.add,
                    accum_out=s1[:])
                xsq = sp_xsq.tile([P, DM], F32, name="xsq_t", tag="xsq_t")
                s2 = sp_s.tile([P, 1], F32, name="s2_t", tag="s2_t")
                nc.scalar.activation(xsq[:], x[:], AF.Square,
                                     accum_out=s2[:])
                mu = sp_s.tile([P, 1], F32, name="mu_t", tag="mu_t")
                nc.vector.tensor_scalar(mu[:], s1[:], 1.0 / DM, None, OP.mult)
                mu2 = sp_s.tile([P, 1], F32, name="mu2_t", tag="mu2_t")
                nc.vector.tensor_mul(mu2[:], mu[:], mu[:])
                va = sp_s.tile([P, 1], F32, name="va_t", tag="va_t")
                nc.vector.scalar_tensor_tensor(
                    va[:], s2[:], 1.0 / DM, mu2[:], OP.mult, OP.subtract)
                vb = sp_s.tile([P, 1], F32, name="vb_t", tag="vb_t")
                nc.vector.tensor_scalar(vb[:], va[:], EPS, None, OP.add)
                sq = sp_s.tile([P, 1], F32, name="sq_t", tag="sq_t")
                nc.scalar.activation(sq[:], vb[:], AF.Sqrt)
                rs = sp_s.tile([P, 1], F32, name="rs_t", tag="rs_t")
                nc.vector.reciprocal(rs[:], sq[:])
                mr = sp_s.tile([P, 1], F32, name="mr_t", tag="mr_t")
                nc.vector.tensor_mul(mr[:], mu[:], rs[:])
                xo = sp_o.tile([P, DM], F32, name="xo_t", tag="xo_t")
                nc.vector.tensor_scalar(
                    xo[:], x[:], rs[:], mr[:], OP.mult, OP.subtract)
                if lng is not None:
                    xg = sp_o.tile([P, DM], F32, name="xg_t", tag="xo_t")
                    nc.vector.tensor_mul(xg[:], xo[:], lng_sb[:])
                    nc.vector.tensor_add(xg[:], xg[:], lnb_sb[:])
                    xo = xg
                nc.sync.dma_start(out[b, tcn * P:(tcn + 1) * P, :], xo[:])

        import os
        nb = int(os.environ.get("KERNEL_NB", str(BC)))
        emit_qkv(0)
        for b in range(nb):
            if b + 1 < nb:
                emit_qkv(b + 1)
            emit_attn(b)


_PROGRAM_CACHE = {}


def _get_program(apply_affine: bool):
    if apply_affine not in _PROGRAM_CACHE:
        _PROGRAM_CACHE[apply_affine] = build_program(apply_affine)
    return _PROGRAM_CACHE[apply_affine]


def make_in_maps(h, qkv_w, o_w, pi0, pi1, ln_g, ln_b, apply_affine):
    h = np.ascontiguousarray(np.asarray(h, np.float32))
    wTn = np.ascontiguousarray(np.asarray(qkv_w, np.float32).T)
    owTn = np.ascontiguousarray(np.asarray(o_w, np.float32).T)
    g0 = np.clip(np.asarray(pi0, np.float32), 0.0, 1.0)   # [NH, L]
    g1 = np.clip(np.asarray(pi1, np.float32), 0.0, 1.0)
    gn = np.zeros((P, 32), np.float32)
    for tcn in range(2):
        for hh in range(NH):
            gn[:, tcn * 16 + hh * 2 + 0] = g0[hh, tcn * P:(tcn + 1) * P]
            gn[:, tcn * 16 + hh * 2 + 1] = g1[hh, tcn * P:(tcn + 1) * P]
    hT_all = np.ascontiguousarray(h.transpose(1, 2, 0))    # [B, DM, L]
    hres_all = np.ascontiguousarray(h.transpose(1, 0, 2))  # [B, L, DM]
    tri_np, ones_np, ident_np = _consts()
    in_maps = []
    for c in range(NC):
        m = {
            "hT": hT_all[c * BC:(c + 1) * BC],
            "hres": hres_all[c * BC:(c + 1) * BC],
            "wT": wTn,
            "owT": owTn,
            "gates": gn,
            "tri_c": tri_np,
            "ones_c": ones_np,
            "ident_c": ident_np,
        }
        if apply_affine:
            m["lng"] = np.ascontiguousarray(
                np.broadcast_to(np.asarray(ln_g, np.float32), (P, DM)))
            m["lnb"] = np.ascontiguousarray(
                np.broadcast_to(np.asarray(ln_b, np.float32), (P, DM)))
        in_maps.append(m)
    return in_maps


def assemble_output(results):
    outs = np.stack([np.asarray(r["out"]) for r in results])  # [NC,BC,L,DM]
    return np.ascontiguousarray(
        outs.reshape(B, L, DM).transpose(1, 0, 2))            # [L, B, DM]


_LAST_RESULTS = None


def kernel(h, qkv_w, o_w, pi0, pi1, ln_g, ln_b, **run_kwargs):
    global _LAST_RESULTS
    ln_g = np.asarray(ln_g, np.float32)
    ln_b = np.asarray(ln_b, np.float32)
    apply_affine = not (np.all(ln_g == 1.0) and np.all(ln_b == 0.0))
    ncb = _get_program(apply_affine)
    in_maps = make_in_maps(h, qkv_w, o_w, pi0, pi1, ln_g, ln_b, apply_affine)
    res = run_bass_kernel_spmd(ncb, in_maps, list(range(NC)), **run_kwargs)
    _LAST_RESULTS = res
    return assemble_output(res.results)
